# revision 10
# baseline (speedup 1.0000x reference)
"""BagOfWordsMLP on 8 Trainium2 NeuronCores.

Strategy (data-parallel, batch-sharded 128 rows/core):
  h1[b,:] = sum_s W1[x[b,s],:] + b1  -- an embedding-bag. Instead of
  materializing the [B, 50257] bag-of-words histogram, each core
  dma_gathers the bf16 W1 rows for its ~36.6K distinct tokens (2KB each)
  and accumulates them into PSUM with PE matmuls whose stationary
  operand carries each row's token multiplicities (built host-side as
  part of input sharding). Bias terms are folded in as K=1 matmuls.
  fc2/fc3 run per-core on the PE after an on-chip transpose. No
  collectives needed.

Vocab is split at 32768 (int16 gather-index limit) into two DRAM
tables; tokens are routed host-side to the matching gather stream.
"""

import sys

import numpy as np

sys.path.insert(0, "/opt/trn_rl_repo")

import ml_dtypes  # noqa: E402

from concourse import bacc, bass, mybir, tile  # noqa: E402,F401
from concourse.bass_utils import run_bass_kernel_spmd  # noqa: E402

BF16 = ml_dtypes.bfloat16

N_CORES = 8
B, S = 1024, 512
B_LOC = B // N_CORES  # 128 rows per core
V = 50257
H1, H2, C = 1024, 512, 20

VSPLIT = 32768
VA_ROWS = VSPLIT  # 32768 rows in table A
VB_ROWS = V - VSPLIT  # 17489 rows in table B

GI = 1024  # gather indices per dma_gather instruction
# Tokens are deduplicated per core (~36.6K unique of 65536); gather slots
# sized mean + ~9 sigma for uniform token draws.
NA = 24  # A-stream gather instructions (24576 slots, mean unique ~23878)
NB = 13  # B-stream gather instructions (13312 slots, mean unique ~12743)
A_CAP = NA * GI
B_CAP = NB * GI
NT = NA + NB  # 37 gather instructions
NST = NT * 8  # 296 matmul subtiles of 128 slots

LAST_EXEC_NS = None


def _build_program():
    nc = bacc.Bacc(
        "TRN2", target_bir_lowering=False, debug=False, num_devices=N_CORES
    )
    f32 = mybir.dt.float32
    bf16 = mybir.dt.bfloat16
    i16 = mybir.dt.int16

    w1a = nc.declare_dram_parameter("w1a", [VA_ROWS, H1], bf16, isOutput=False)
    w1b = nc.declare_dram_parameter("w1b", [VB_ROWS, H1], bf16, isOutput=False)
    idxa = nc.declare_dram_parameter("idxa", [NA, 128, GI // 16], i16, isOutput=False)
    idxb = nc.declare_dram_parameter("idxb", [NB, 128, GI // 16], i16, isOutput=False)
    oh = nc.declare_dram_parameter("oh", [NST, 128, 128], bf16, isOutput=False)
    w2 = nc.declare_dram_parameter("w2", [H1, H2], bf16, isOutput=False)
    wout = nc.declare_dram_parameter("wout", [H2, C], bf16, isOutput=False)
    b1 = nc.declare_dram_parameter("b1", [1, H1], bf16, isOutput=False)
    b2 = nc.declare_dram_parameter("b2", [1, H2], bf16, isOutput=False)
    bo = nc.declare_dram_parameter("bo", [1, C], bf16, isOutput=False)
    ident = nc.declare_dram_parameter("ident", [128, 128], bf16, isOutput=False)
    ones1 = nc.declare_dram_parameter("ones1", [1, 128], bf16, isOutput=False)
    out_d = nc.declare_dram_parameter("out", [B_LOC, C], f32, isOutput=True)

    with tile.TileContext(nc) as tc:
        with (
            tc.tile_pool(name="wpool", bufs=1) as wpool,
            tc.tile_pool(name="gpool", bufs=3) as gpool,
            tc.tile_pool(name="ohpool", bufs=3) as ohpool,
            tc.tile_pool(name="ipool", bufs=3) as ipool,
            tc.tile_pool(name="hpool", bufs=1) as hpool,
            tc.tile_pool(name="acc", bufs=1, space="PSUM") as accpool,
            tc.tile_pool(name="tpp", bufs=2, space="PSUM") as tppool,
        ):
            # --- stage small weights ---
            w2_sb = wpool.tile([128, H1 // 128, H2], bf16)
            nc.sync.dma_start(
                out=w2_sb[:], in_=w2.rearrange("(c p) n -> p c n", p=128)
            )
            wout_sb = wpool.tile([128, H2 // 128, C], bf16)
            nc.sync.dma_start(
                out=wout_sb[:], in_=wout.rearrange("(c p) n -> p c n", p=128)
            )
            b1_sb = wpool.tile([1, H1], bf16)
            nc.sync.dma_start(out=b1_sb[:], in_=b1[:])
            b2_sb = wpool.tile([1, H2], bf16)
            nc.sync.dma_start(out=b2_sb[:], in_=b2[:])
            bo_sb = wpool.tile([1, C], bf16)
            nc.sync.dma_start(out=bo_sb[:], in_=bo[:])
            id_sb = wpool.tile([128, 128], bf16)
            nc.sync.dma_start(out=id_sb[:], in_=ident[:])
            on_sb = wpool.tile([1, 128], bf16)
            nc.sync.dma_start(out=on_sb[:], in_=ones1[:])

            # --- fc1: embedding-bag accumulation into PSUM ---
            p_lo = accpool.tile([128, 512], f32)
            p_hi = accpool.tile([128, 512], f32)
            # bias seeds the accumulator (K=1 matmul: ones^T @ b1 slice)
            nc.tensor.matmul(
                p_lo[:], on_sb[:], b1_sb[:, 0:512], start=True, stop=False
            )
            nc.tensor.matmul(
                p_hi[:], on_sb[:], b1_sb[:, 512:1024], start=True, stop=False
            )

            for t in range(NT):
                if t < NA:
                    src, idx_src = w1a, idxa[t]
                else:
                    src, idx_src = w1b, idxb[t - NA]
                it = ipool.tile([128, GI // 16], i16)
                nc.sync.dma_start(out=it[:], in_=idx_src)
                g = gpool.tile([128, 8, H1], bf16)
                nc.gpsimd.dma_gather(
                    g[:],
                    src[:],
                    it[:],
                    num_idxs=GI,
                    num_idxs_reg=GI,
                    elem_size=H1,
                )
                oht = ohpool.tile([128, 8, 128], bf16, tag="oht")
                st0 = t * 8
                nc.sync.dma_start(
                    out=oht[:],
                    in_=oh[st0 : st0 + 8].rearrange("s p m -> p s m"),
                )
                for c in range(8):
                    last = t == NT - 1 and c == 7
                    oc = c
                    nc.tensor.matmul(
                        p_lo[:],
                        oht[:, oc, :],
                        g[:, c, 0:512],
                        start=False,
                        stop=last,
                    )
                    nc.tensor.matmul(
                        p_hi[:],
                        oht[:, oc, :],
                        g[:, c, 512:1024],
                        start=False,
                        stop=last,
                    )

            # --- h1 = relu(psum) -> bf16 ---
            h1 = hpool.tile([128, H1], bf16)
            nc.scalar.activation(
                h1[:, 0:512], p_lo[:], mybir.ActivationFunctionType.Relu
            )
            nc.scalar.activation(
                h1[:, 512:1024], p_hi[:], mybir.ActivationFunctionType.Relu
            )

            # --- transpose h1 -> h1T chunks [hid_local, row] ---
            h1t = hpool.tile([128, H1 // 128, 128], bf16)
            for cix in range(H1 // 128):
                tp = tppool.tile([128, 128], bf16)
                nc.tensor.transpose(
                    tp[:], h1[:, cix * 128 : (cix + 1) * 128], id_sb[:]
                )
                nc.scalar.activation(
                    h1t[:, cix, :], tp[:], mybir.ActivationFunctionType.Copy
                )

            # --- fc2 ---
            p_h2 = accpool.tile([128, H2], f32)
            nc.tensor.matmul(p_h2[:], on_sb[:], b2_sb[:], start=True, stop=False)
            for cix in range(H1 // 128):
                nc.tensor.matmul(
                    p_h2[:],
                    h1t[:, cix, :],
                    w2_sb[:, cix, :],
                    start=False,
                    stop=(cix == H1 // 128 - 1),
                )
            h2 = hpool.tile([128, H2], bf16)
            nc.scalar.activation(h2[:], p_h2[:], mybir.ActivationFunctionType.Relu)

            # --- transpose h2 ---
            h2t = hpool.tile([128, H2 // 128, 128], bf16)
            for cix in range(H2 // 128):
                tp = tppool.tile([128, 128], bf16)
                nc.tensor.transpose(
                    tp[:], h2[:, cix * 128 : (cix + 1) * 128], id_sb[:]
                )
                nc.scalar.activation(
                    h2t[:, cix, :], tp[:], mybir.ActivationFunctionType.Copy
                )

            # --- fc3 ---
            p_out = accpool.tile([128, C], f32)
            nc.tensor.matmul(p_out[:], on_sb[:], bo_sb[:], start=True, stop=False)
            for cix in range(H2 // 128):
                nc.tensor.matmul(
                    p_out[:],
                    h2t[:, cix, :],
                    wout_sb[:, cix, :],
                    start=False,
                    stop=(cix == H2 // 128 - 1),
                )
            o_sb = hpool.tile([128, C], f32)
            nc.vector.tensor_copy(o_sb[:], p_out[:])
            nc.sync.dma_start(out=out_d[:], in_=o_sb[:])

    nc.compile()
    return nc


def _shard_inputs(x, W1, b1v, W2, b2v, Wout, boutv):
    x = np.asarray(x).astype(np.int64)
    assert x.shape == (B, S), x.shape
    w1a = np.ascontiguousarray(np.asarray(W1, dtype=np.float32)[:VSPLIT]).astype(BF16)
    w1b = np.ascontiguousarray(np.asarray(W1, dtype=np.float32)[VSPLIT:]).astype(BF16)
    w2 = np.asarray(W2, dtype=np.float32).astype(BF16)
    wout = np.asarray(Wout, dtype=np.float32).astype(BF16)
    b1a = np.asarray(b1v, dtype=np.float32).astype(BF16).reshape(1, H1)
    b2a = np.asarray(b2v, dtype=np.float32).astype(BF16).reshape(1, H2)
    boa = np.asarray(boutv, dtype=np.float32).astype(BF16).reshape(1, C)
    identity = np.eye(128, dtype=np.float32).astype(BF16)
    ones1 = np.ones((1, 128), dtype=np.float32).astype(BF16)

    in_maps = []
    for k in range(N_CORES):
        tokens = x[k * B_LOC : (k + 1) * B_LOC].reshape(-1)
        rows = np.arange(tokens.size, dtype=np.int64) // S
        # Dedup across the core's 128 rows: gather each distinct W1 row once,
        # weight it by its per-row multiplicity in the stationary operand.
        uv, inv = np.unique(tokens, return_inverse=True)
        cnt = np.zeros((uv.size, B_LOC), dtype=np.float32)
        np.add.at(cnt, (inv, rows), 1.0)
        assert cnt.max() <= 256  # bf16-exact integer range
        a_sel = uv < VSPLIT
        a_vals, a_cnt = uv[a_sel], cnt[a_sel]
        b_vals, b_cnt = uv[~a_sel] - VSPLIT, cnt[~a_sel]
        assert a_vals.size <= A_CAP, a_vals.size
        assert b_vals.size <= B_CAP, b_vals.size

        def pack(vals, cm, cap, nt):
            v = np.zeros(cap, dtype=np.int16)
            c = np.zeros((cap, B_LOC), dtype=np.float32)
            v[: vals.size] = vals.astype(np.int16)
            c[: vals.size] = cm
            # idx layout: element j of instr t lives at [t, p, j//16] for
            # p % 16 == j % 16 (replicated across the 8 partition groups)
            arr = v.reshape(nt, GI // 16, 16).transpose(0, 2, 1)
            arr = np.ascontiguousarray(np.tile(arr, (1, 8, 1)))
            return arr, c

        idxa_arr, a_cnt_p = pack(a_vals, a_cnt, A_CAP, NA)
        idxb_arr, b_cnt_p = pack(b_vals, b_cnt, B_CAP, NB)

        ohm = (
            np.concatenate([a_cnt_p, b_cnt_p])
            .reshape(NST, 128, 128)
            .astype(BF16)
        )

        in_maps.append(
            {
                "w1a": w1a,
                "w1b": w1b,
                "idxa": idxa_arr,
                "idxb": idxb_arr,
                "oh": ohm,
                "w2": w2,
                "wout": wout,
                "b1": b1a,
                "b2": b2a,
                "bo": boa,
                "ident": identity,
                "ones1": ones1,
            }
        )
    return in_maps


_NC_CACHE = None


def modeled_exec_ns():
    """Cost-model (TimelineSim) per-core execution time for the program.

    The axon client in this container has no NTFF profiling hook, so this
    is the best available per-core HW-time estimate.
    """
    global _NC_CACHE
    if _NC_CACHE is None:
        _NC_CACHE = _build_program()
    from concourse.timeline_sim import TimelineSim

    return TimelineSim(_NC_CACHE, trace=False).simulate()


def kernel(x, W1, b1, W2, b2, Wout, bout):
    global _NC_CACHE, LAST_EXEC_NS
    in_maps = _shard_inputs(x, W1, b1, W2, b2, Wout, bout)
    if _NC_CACHE is None:
        _NC_CACHE = _build_program()
    res = run_bass_kernel_spmd(_NC_CACHE, in_maps, list(range(N_CORES)))
    LAST_EXEC_NS = res.exec_time_ns
    out = np.concatenate(
        [np.asarray(res.results[k]["out"]) for k in range(N_CORES)], axis=0
    )
    return out.astype(np.float32)


if __name__ == "__main__":
    rng = np.random.default_rng(0)
    x = rng.integers(0, V, size=(B, S), dtype=np.int64)
    W1 = rng.standard_normal((V, H1), dtype=np.float32) * 0.004
    b1v = rng.standard_normal(H1, dtype=np.float32) * 0.004
    W2 = rng.standard_normal((H1, H2), dtype=np.float32) * 0.03
    b2v = rng.standard_normal(H2, dtype=np.float32) * 0.03
    Wout = rng.standard_normal((H2, C), dtype=np.float32) * 0.04
    bov = rng.standard_normal(C, dtype=np.float32) * 0.04
    got = kernel(x, W1, b1v, W2, b2v, Wout, bov)
    bow = np.zeros((B, V), dtype=np.float32)
    np.add.at(bow, (np.repeat(np.arange(B), S), x.reshape(-1)), 1.0)
    h = np.maximum(bow @ W1 + b1v, 0)
    h = np.maximum(h @ W2 + b2v, 0)
    want = h @ Wout + bov
    err = np.abs(got - want).max() / (np.abs(want).max() + 1e-9)
    print("rel err:", err)


# revision 11
# speedup vs baseline: 1.1203x; 1.1203x over previous
"""BagOfWordsMLP on 8 Trainium2 NeuronCores.

Strategy (data-parallel, batch-sharded 128 rows/core):
  h1[b,:] = sum_s W1[x[b,s],:] + b1  -- an embedding-bag. Instead of
  materializing the [B, 50257] bag-of-words histogram, each core
  dma_gathers the bf16 W1 rows for its ~36.6K distinct tokens (2KB each)
  and accumulates them into PSUM with PE matmuls whose stationary
  operand carries each row's token multiplicities (built host-side as
  part of input sharding). Bias terms are folded in as K=1 matmuls.
  fc2/fc3 run per-core on the PE after an on-chip transpose. No
  collectives needed.

Vocab is split at 32768 (int16 gather-index limit) into two DRAM
tables; tokens are routed host-side to the matching gather stream.
"""

import sys

import numpy as np

sys.path.insert(0, "/opt/trn_rl_repo")

import ml_dtypes  # noqa: E402

from concourse import bacc, bass, mybir, tile  # noqa: E402,F401
from concourse.bass_utils import run_bass_kernel_spmd  # noqa: E402

BF16 = ml_dtypes.bfloat16

N_CORES = 8
B, S = 1024, 512
B_LOC = B // N_CORES  # 128 rows per core
V = 50257
H1, H2, C = 1024, 512, 20

VSPLIT = 32768
VA_ROWS = VSPLIT  # 32768 rows in table A
VB_ROWS = V - VSPLIT  # 17489 rows in table B

GI = 1024  # gather indices per dma_gather instruction
# Tokens are deduplicated per core (~36.6K unique of 65536); gather slots
# sized mean + ~9 sigma for uniform token draws.
NA = 24  # A-stream gather instructions (24576 slots, mean unique ~23878)
NB = 13  # B-stream gather instructions (13312 slots, mean unique ~12743)
A_CAP = NA * GI
B_CAP = NB * GI
NT = NA + NB  # 37 gather instructions
NST = NT * 8  # 296 matmul subtiles of 128 slots

LAST_EXEC_NS = None


def _build_program():
    nc = bacc.Bacc(
        "TRN2", target_bir_lowering=False, debug=False, num_devices=N_CORES
    )
    f32 = mybir.dt.float32
    bf16 = mybir.dt.bfloat16
    i16 = mybir.dt.int16

    w1a = nc.declare_dram_parameter("w1a", [VA_ROWS, H1], bf16, isOutput=False)
    w1b = nc.declare_dram_parameter("w1b", [VB_ROWS, H1], bf16, isOutput=False)
    idxa = nc.declare_dram_parameter("idxa", [NA, 128, GI // 16], i16, isOutput=False)
    idxb = nc.declare_dram_parameter("idxb", [NB, 128, GI // 16], i16, isOutput=False)
    oh = nc.declare_dram_parameter("oh", [NST, 128, 128], mybir.dt.float8e4, isOutput=False)
    w2 = nc.declare_dram_parameter("w2", [H1, H2], bf16, isOutput=False)
    wout = nc.declare_dram_parameter("wout", [H2, C], bf16, isOutput=False)
    b1 = nc.declare_dram_parameter("b1", [1, H1], bf16, isOutput=False)
    b2 = nc.declare_dram_parameter("b2", [1, H2], bf16, isOutput=False)
    bo = nc.declare_dram_parameter("bo", [1, C], bf16, isOutput=False)
    ident = nc.declare_dram_parameter("ident", [128, 128], bf16, isOutput=False)
    ones1 = nc.declare_dram_parameter("ones1", [1, 128], bf16, isOutput=False)
    out_d = nc.declare_dram_parameter("out", [B_LOC, C], f32, isOutput=True)

    with tile.TileContext(nc) as tc:
        with (
            tc.tile_pool(name="wpool", bufs=1) as wpool,
            tc.tile_pool(name="gpool", bufs=3) as gpool,
            tc.tile_pool(name="ohpool", bufs=3) as ohpool,
            tc.tile_pool(name="ipool", bufs=3) as ipool,
            tc.tile_pool(name="hpool", bufs=1) as hpool,
            tc.tile_pool(name="acc", bufs=1, space="PSUM") as accpool,
            tc.tile_pool(name="tpp", bufs=2, space="PSUM") as tppool,
        ):
            # --- stage small weights ---
            w2_sb = wpool.tile([128, H1 // 128, H2], bf16)
            nc.sync.dma_start(
                out=w2_sb[:], in_=w2.rearrange("(c p) n -> p c n", p=128)
            )
            wout_sb = wpool.tile([128, H2 // 128, C], bf16)
            nc.sync.dma_start(
                out=wout_sb[:], in_=wout.rearrange("(c p) n -> p c n", p=128)
            )
            b1_sb = wpool.tile([1, H1], bf16)
            nc.sync.dma_start(out=b1_sb[:], in_=b1[:])
            b2_sb = wpool.tile([1, H2], bf16)
            nc.sync.dma_start(out=b2_sb[:], in_=b2[:])
            bo_sb = wpool.tile([1, C], bf16)
            nc.sync.dma_start(out=bo_sb[:], in_=bo[:])
            id_sb = wpool.tile([128, 128], bf16)
            nc.sync.dma_start(out=id_sb[:], in_=ident[:])
            on_sb = wpool.tile([1, 128], bf16)
            nc.sync.dma_start(out=on_sb[:], in_=ones1[:])

            # --- fc1: embedding-bag accumulation into PSUM ---
            p_lo = accpool.tile([128, 512], f32)
            p_hi = accpool.tile([128, 512], f32)
            # bias seeds the accumulator (K=1 matmul: ones^T @ b1 slice)
            nc.tensor.matmul(
                p_lo[:], on_sb[:], b1_sb[:, 0:512], start=True, stop=False
            )
            nc.tensor.matmul(
                p_hi[:], on_sb[:], b1_sb[:, 512:1024], start=True, stop=False
            )

            for t in range(NT):
                if t < NA:
                    src, idx_src = w1a, idxa[t]
                else:
                    src, idx_src = w1b, idxb[t - NA]
                it = ipool.tile([128, GI // 16], i16)
                nc.sync.dma_start(out=it[:], in_=idx_src)
                g = gpool.tile([128, 8, H1], bf16)
                nc.gpsimd.dma_gather(
                    g[:],
                    src[:],
                    it[:],
                    num_idxs=GI,
                    num_idxs_reg=GI,
                    elem_size=H1,
                )
                oht = ohpool.tile([128, 8, 128], mybir.dt.float8e4, tag="oht")
                st0 = t * 8
                nc.sync.dma_start(
                    out=oht[:],
                    in_=oh[st0 : st0 + 8].rearrange("s p m -> p s m"),
                )
                for c in range(8):
                    last = t == NT - 1 and c == 7
                    oc = c
                    nc.tensor.matmul(
                        p_lo[:],
                        oht[:, oc, :],
                        g[:, c, 0:512],
                        start=False,
                        stop=last,
                    )
                    nc.tensor.matmul(
                        p_hi[:],
                        oht[:, oc, :],
                        g[:, c, 512:1024],
                        start=False,
                        stop=last,
                    )

            # --- h1 = relu(psum) -> bf16 ---
            h1 = hpool.tile([128, H1], bf16)
            nc.scalar.activation(
                h1[:, 0:512], p_lo[:], mybir.ActivationFunctionType.Relu
            )
            nc.scalar.activation(
                h1[:, 512:1024], p_hi[:], mybir.ActivationFunctionType.Relu
            )

            # --- transpose h1 -> h1T chunks [hid_local, row] ---
            h1t = hpool.tile([128, H1 // 128, 128], bf16)
            for cix in range(H1 // 128):
                tp = tppool.tile([128, 128], bf16)
                nc.tensor.transpose(
                    tp[:], h1[:, cix * 128 : (cix + 1) * 128], id_sb[:]
                )
                nc.scalar.activation(
                    h1t[:, cix, :], tp[:], mybir.ActivationFunctionType.Copy
                )

            # --- fc2 ---
            p_h2 = accpool.tile([128, H2], f32)
            nc.tensor.matmul(p_h2[:], on_sb[:], b2_sb[:], start=True, stop=False)
            for cix in range(H1 // 128):
                nc.tensor.matmul(
                    p_h2[:],
                    h1t[:, cix, :],
                    w2_sb[:, cix, :],
                    start=False,
                    stop=(cix == H1 // 128 - 1),
                )
            h2 = hpool.tile([128, H2], bf16)
            nc.scalar.activation(h2[:], p_h2[:], mybir.ActivationFunctionType.Relu)

            # --- transpose h2 ---
            h2t = hpool.tile([128, H2 // 128, 128], bf16)
            for cix in range(H2 // 128):
                tp = tppool.tile([128, 128], bf16)
                nc.tensor.transpose(
                    tp[:], h2[:, cix * 128 : (cix + 1) * 128], id_sb[:]
                )
                nc.scalar.activation(
                    h2t[:, cix, :], tp[:], mybir.ActivationFunctionType.Copy
                )

            # --- fc3 ---
            p_out = accpool.tile([128, C], f32)
            nc.tensor.matmul(p_out[:], on_sb[:], bo_sb[:], start=True, stop=False)
            for cix in range(H2 // 128):
                nc.tensor.matmul(
                    p_out[:],
                    h2t[:, cix, :],
                    wout_sb[:, cix, :],
                    start=False,
                    stop=(cix == H2 // 128 - 1),
                )
            o_sb = hpool.tile([128, C], f32)
            nc.vector.tensor_copy(o_sb[:], p_out[:])
            nc.sync.dma_start(out=out_d[:], in_=o_sb[:])

    nc.compile()
    return nc


def _shard_inputs(x, W1, b1v, W2, b2v, Wout, boutv):
    x = np.asarray(x).astype(np.int64)
    assert x.shape == (B, S), x.shape
    w1a = np.ascontiguousarray(np.asarray(W1, dtype=np.float32)[:VSPLIT]).astype(BF16)
    w1b = np.ascontiguousarray(np.asarray(W1, dtype=np.float32)[VSPLIT:]).astype(BF16)
    w2 = np.asarray(W2, dtype=np.float32).astype(BF16)
    wout = np.asarray(Wout, dtype=np.float32).astype(BF16)
    b1a = np.asarray(b1v, dtype=np.float32).astype(BF16).reshape(1, H1)
    b2a = np.asarray(b2v, dtype=np.float32).astype(BF16).reshape(1, H2)
    boa = np.asarray(boutv, dtype=np.float32).astype(BF16).reshape(1, C)
    identity = np.eye(128, dtype=np.float32).astype(BF16)
    ones1 = np.ones((1, 128), dtype=np.float32).astype(BF16)

    in_maps = []
    for k in range(N_CORES):
        tokens = x[k * B_LOC : (k + 1) * B_LOC].reshape(-1)
        rows = np.arange(tokens.size, dtype=np.int64) // S
        # Dedup across the core's 128 rows: gather each distinct W1 row once,
        # weight it by its per-row multiplicity in the stationary operand.
        uv, inv = np.unique(tokens, return_inverse=True)
        cnt = np.zeros((uv.size, B_LOC), dtype=np.float32)
        np.add.at(cnt, (inv, rows), 1.0)
        assert cnt.max() <= 256  # bf16-exact integer range
        a_sel = uv < VSPLIT
        a_vals, a_cnt = uv[a_sel], cnt[a_sel]
        b_vals, b_cnt = uv[~a_sel] - VSPLIT, cnt[~a_sel]
        assert a_vals.size <= A_CAP, a_vals.size
        assert b_vals.size <= B_CAP, b_vals.size

        def pack(vals, cm, cap, nt):
            v = np.zeros(cap, dtype=np.int16)
            c = np.zeros((cap, B_LOC), dtype=np.float32)
            v[: vals.size] = vals.astype(np.int16)
            c[: vals.size] = cm
            # idx layout: element j of instr t lives at [t, p, j//16] for
            # p % 16 == j % 16 (replicated across the 8 partition groups)
            arr = v.reshape(nt, GI // 16, 16).transpose(0, 2, 1)
            arr = np.ascontiguousarray(np.tile(arr, (1, 8, 1)))
            return arr, c

        idxa_arr, a_cnt_p = pack(a_vals, a_cnt, A_CAP, NA)
        idxb_arr, b_cnt_p = pack(b_vals, b_cnt, B_CAP, NB)

        assert cnt.max() <= 16  # fp8 e4m3 exact-integer range
        ohm = (
            np.concatenate([a_cnt_p, b_cnt_p])
            .reshape(NST, 128, 128)
            .astype(ml_dtypes.float8_e4m3)
        )

        in_maps.append(
            {
                "w1a": w1a,
                "w1b": w1b,
                "idxa": idxa_arr,
                "idxb": idxb_arr,
                "oh": ohm,
                "w2": w2,
                "wout": wout,
                "b1": b1a,
                "b2": b2a,
                "bo": boa,
                "ident": identity,
                "ones1": ones1,
            }
        )
    return in_maps


_NC_CACHE = None


def modeled_exec_ns():
    """Cost-model (TimelineSim) per-core execution time for the program.

    The axon client in this container has no NTFF profiling hook, so this
    is the best available per-core HW-time estimate.
    """
    global _NC_CACHE
    if _NC_CACHE is None:
        _NC_CACHE = _build_program()
    from concourse.timeline_sim import TimelineSim

    return TimelineSim(_NC_CACHE, trace=False).simulate()


def kernel(x, W1, b1, W2, b2, Wout, bout):
    global _NC_CACHE, LAST_EXEC_NS
    in_maps = _shard_inputs(x, W1, b1, W2, b2, Wout, bout)
    if _NC_CACHE is None:
        _NC_CACHE = _build_program()
    res = run_bass_kernel_spmd(_NC_CACHE, in_maps, list(range(N_CORES)))
    LAST_EXEC_NS = res.exec_time_ns
    out = np.concatenate(
        [np.asarray(res.results[k]["out"]) for k in range(N_CORES)], axis=0
    )
    return out.astype(np.float32)


if __name__ == "__main__":
    rng = np.random.default_rng(0)
    x = rng.integers(0, V, size=(B, S), dtype=np.int64)
    W1 = rng.standard_normal((V, H1), dtype=np.float32) * 0.004
    b1v = rng.standard_normal(H1, dtype=np.float32) * 0.004
    W2 = rng.standard_normal((H1, H2), dtype=np.float32) * 0.03
    b2v = rng.standard_normal(H2, dtype=np.float32) * 0.03
    Wout = rng.standard_normal((H2, C), dtype=np.float32) * 0.04
    bov = rng.standard_normal(C, dtype=np.float32) * 0.04
    got = kernel(x, W1, b1v, W2, b2v, Wout, bov)
    bow = np.zeros((B, V), dtype=np.float32)
    np.add.at(bow, (np.repeat(np.arange(B), S), x.reshape(-1)), 1.0)
    h = np.maximum(bow @ W1 + b1v, 0)
    h = np.maximum(h @ W2 + b2v, 0)
    want = h @ Wout + bov
    err = np.abs(got - want).max() / (np.abs(want).max() + 1e-9)
    print("rel err:", err)
